# revision 15
# baseline (speedup 1.0000x reference)
"""Trainium2 Bass kernel for nn_BindingAffinityGAT (3x GATv2 + BN + ELU,
global attention pooling, MLP head) on 8 NeuronCores.

Sharding: nodes partitioned by dst across the 8 cores (1250 each); edges
live with their destination core (fixed 125-dst chunks x 18 K-tiles of 128
edge slots, zero-padded so all cores run one SPMD program); per-layer halo
exchange (AllGather) of the source-side transformed features; weights
replicated. Aggregation + segment softmax run as PE matmuls against a
host-built edge->dst one-hot (S) so variable-degree segments never touch
the vector engine.

Self-contained: hardcodes all shapes; host-side numpy does graph prep.
"""

import sys
import types

import numpy as np


def _install_ntff_hook():
    if "antenv.axon_hooks" in sys.modules:
        return
    mod = types.ModuleType("antenv.axon_hooks")
    mod._hook = None
    mod.set_axon_ntff_profile_hook = lambda h: setattr(mod, "_hook", h)
    mod.get_axon_ntff_profile_hook = lambda: mod._hook
    sys.modules["antenv.axon_hooks"] = mod
    import antenv

    antenv.axon_hooks = mod
    try:
        from trn_agent_boot.trn_boot import _ntff_profile_via_ctypes

        mod._hook = _ntff_profile_via_ctypes("/opt/axon/libaxon_pjrt.so")
    except Exception:
        pass


_install_ntff_hook()

import concourse.bacc as bacc
import concourse.mybir as mybir
import concourse.tile as tile
from concourse import library_config
from concourse.bass_utils import run_bass_kernel_spmd

# ---------------------------------------------------------------- constants
N = 10000
E = 160000
IN = 49
H = 8
C = 128
D = H * C  # 1024
B = 16
NEG = 0.2
EPS = 1e-5

NCORES = 8
NLOC = N // NCORES  # 1250
NCHUNK = 10  # dst chunks per core
DCH = NLOC // NCHUNK  # 125 dsts per chunk
KT = 18  # K-tiles (128 edge slots) per chunk; fixed across cores
ESLOT = KT * 128  # 2304 edge slots per chunk
GKT = 2  # K-tiles per gather group
NG = KT // GKT  # gather groups per chunk
F32 = mybir.dt.float32
I16 = mybir.dt.int16
AX = mybir.AxisListType
ALU = mybir.AluOpType
AF = mybir.ActivationFunctionType

_CACHE: dict = {}
import os
BISECT = os.environ.get('KBISECT', '')


# ---------------------------------------------------------------- host prep
def _wrap_idx(lin):
    assert len(lin) % 16 == 0
    w = lin.reshape(-1, 16).T  # [16, S]; linear j -> (j % 16, j // 16)
    return np.tile(w, (8, 1)).astype(np.int16)


def _prep_graph(edge_index):
    src = np.concatenate([edge_index[0], np.arange(N)]).astype(np.int64)
    dst = np.concatenate([edge_index[1], np.arange(N)]).astype(np.int64)
    order = np.argsort(dst, kind="stable")
    src, dst = src[order], dst[order]

    cores = []
    for c in range(NCORES):
        lo, hi = c * NLOC, (c + 1) * NLOC
        m = (dst >= lo) & (dst < hi)
        cs, cd = src[m], dst[m] - lo
        src_slots = np.zeros(NCHUNK * ESLOT, np.int64)
        dst_slots = np.zeros(NCHUNK * ESLOT, np.int64)
        s_mat = np.zeros((NCHUNK * ESLOT, 128), np.float32)
        for k in range(NCHUNK):
            dlo, dhi = k * DCH, (k + 1) * DCH
            km = (cd >= dlo) & (cd < dhi)
            ks, kd = cs[km], cd[km] - dlo
            ne = len(ks)
            assert ne <= ESLOT, f"chunk overflow: {ne} > {ESLOT}"
            base = k * ESLOT
            src_slots[base : base + ne] = ks
            dst_slots[base : base + ne] = kd + dlo
            s_mat[np.arange(base, base + ne), kd] = 1.0
        cores.append(
            dict(
                src_idx=_wrap_idx(src_slots.astype(np.int16)),
                dst_idx=_wrap_idx(dst_slots.astype(np.int16)),
                s_mat=s_mat,
            )
        )
    return cores


def _prep_pool(batch):
    cores = []
    for c in range(NCORES):
        lo = c * NLOC
        tiles = []
        for k in range(NCHUNK):
            oh = np.zeros((128, B), np.float32)
            seg = np.asarray(batch)[lo + k * DCH : lo + (k + 1) * DCH]
            oh[np.arange(DCH), seg] = 1.0
            tiles.append(oh)
        cores.append(np.stack(tiles).astype(np.float32))
    return cores


def _rep128(v):
    return np.tile(np.asarray(v, np.float32).reshape(1, -1), (128, 1))


def _prep_params(params):
    p = {}

    def bn_affine(bn):
        sc = np.asarray(bn["gamma"] / np.sqrt(bn["var"] + EPS), np.float32)
        sh = np.asarray(bn["beta"] - bn["mean"] * sc, np.float32)
        return sc, sh

    for li, gk, bk in ((1, "gat1", "bn1"), (2, "gat2", "bn2"), (3, "gat3", "bn3")):
        g = params[gk]
        p[f"wl{li}"] = np.asarray(g["Wl"], np.float32)
        p[f"wr{li}"] = np.asarray(g["Wr"], np.float32)
        p[f"blb{li}"] = np.asarray(g["bl"], np.float32).reshape(1, D)
        p[f"brb{li}"] = np.asarray(g["br"], np.float32).reshape(1, D)
        att = np.asarray(g["att"], np.float32)
        p[f"att_bc{li}"] = _rep128(att.reshape(D))
        sc, sh = bn_affine(params[bk])
        bias = np.asarray(g["bias"], np.float32)
        if li < 3:
            p[f"bnsc{li}"] = _rep128(sc)
            p[f"bnsh{li}"] = _rep128(sh + bias * sc)
        else:
            p[f"bnsc{li}"] = _rep128(sc / H)  # fold mean-over-heads
            p[f"bnsh{li}"] = _rep128(sh + bias * sc)
    # device h := elu+1 (>=0); fold the -1 into consumers' biases
    for li in (2, 3):
        for side in ("l", "r"):
            w = p[f"w{side}{li}"]
            p[f"b{side}b{li}"] = p[f"b{side}b{li}"] - w.sum(0).reshape(1, D)
    gw1 = np.asarray(params["gate_w1"], np.float32)
    gb1 = np.asarray(params["gate_b1"], np.float32).reshape(1, -1) - gw1.sum(0)
    gw2 = np.asarray(params["gate_w2"], np.float32)
    fw1 = np.asarray(params["fc1_w"], np.float32)
    fb1 = np.asarray(params["fc1_b"], np.float32).reshape(1, -1) - fw1.sum(0)
    p.update(
        gw1=gw1, gb1=gb1, gw2=gw2, fw1=fw1, fb1=fb1,
        fw2=np.asarray(params["fc2_w"], np.float32),
        fb2=np.asarray(params["fc2_b"], np.float32).reshape(1, -1),
        fw3=np.asarray(params["fc3_w"], np.float32),
        fb3=np.asarray(params["fc3_b"], np.float32).reshape(1, -1),
        ident=np.eye(128, dtype=np.float32),
        ones_col=np.ones((128, 1), np.float32),
        ones_lh=np.ones((1, 128), np.float32),
    )
    return p


# ---------------------------------------------------------------- device build
_WSHAPES = {
    "wl1": [IN, D], "wr1": [IN, D], "blb1": [1, D], "brb1": [1, D],
    "wl2": [D, D], "wr2": [D, D], "blb2": [1, D], "brb2": [1, D],
    "wl3": [D, D], "wr3": [D, D], "blb3": [1, D], "brb3": [1, D],
    "att_bc1": [128, D], "bnsc1": [128, D], "bnsh1": [128, D],
    "att_bc2": [128, D], "bnsc2": [128, D], "bnsh2": [128, D],
    "att_bc3": [128, D], "bnsc3": [128, C], "bnsh3": [128, C],
    "gw1": [C, C // 2], "gb1": [1, C // 2], "gw2": [C // 2, 1],
    "fw1": [C, 128], "fb1": [1, 128], "fw2": [128, 64], "fb2": [1, 64],
    "fw3": [64, 1], "fb3": [1, 1], "ident": [128, 128],
    "ones_col": [128, 1], "ones_lh": [1, 128],
}


def _build():
    nc = bacc.Bacc("TRN2", target_bir_lowering=False, debug=False, num_devices=NCORES)

    xT = nc.declare_dram_parameter("xT", [IN, N], F32, isOutput=False)
    xTl = nc.declare_dram_parameter("xTl", [IN, NLOC], F32, isOutput=False)
    src_idx = nc.declare_dram_parameter(
        "src_idx", [128, NCHUNK * ESLOT // 16], I16, isOutput=False
    )
    dst_idx = nc.declare_dram_parameter(
        "dst_idx", [128, NCHUNK * ESLOT // 16], I16, isOutput=False
    )
    s_mat = nc.declare_dram_parameter("s_mat", [NCHUNK * ESLOT, 128], F32, isOutput=False)
    pool_oh = nc.declare_dram_parameter("pool_oh", [NCHUNK, 128, B], F32, isOutput=False)
    wp = {
        name: nc.declare_dram_parameter(name, shape, F32, isOutput=False)
        for name, shape in _WSHAPES.items()
    }
    out_y = nc.declare_dram_parameter("y", [B, 1], F32, isOutput=True)

    vl_tab1 = nc.dram_tensor("vl_tab1", [N, D], F32)
    vr_tab = [nc.dram_tensor(f"vr_tab{li}", [NLOC, D], F32) for li in (1, 2, 3)]
    vl_bounce = [None,
                 nc.dram_tensor("vl_b2", [NLOC, D], F32),
                 nc.dram_tensor("vl_b3", [NLOC, D], F32)]
    vl_shared = [None,
                 nc.dram_tensor("vl_s2", [N, D], F32, addr_space="Shared"),
                 nc.dram_tensor("vl_s3", [N, D], F32, addr_space="Shared")]
    vl_tab = [vl_tab1, vl_shared[1], vl_shared[2]]
    pool_bounce = nc.dram_tensor("pool_bounce", [B, C + 1], F32)
    pool_shared = nc.dram_tensor("pool_shared", [B, C + 1], F32, addr_space="Shared")

    with tile.TileContext(nc) as tc:
        nc.gpsimd.load_library(library_config.mlp)
        _emit(nc, tc, xT, xTl, src_idx, dst_idx, s_mat, pool_oh, wp,
              vl_tab, vr_tab, vl_bounce, vl_shared, pool_bounce, pool_shared,
              out_y)
    nc.compile()
    return nc


def _emit(nc, tc, xT, xTl, src_idx, dst_idx, s_mat, pool_oh, wp,
          vl_tab, vr_tab, vl_bounce, vl_shared, pool_bounce, pool_shared,
          out_y):
    with (
        tc.tile_pool(name="wbig", bufs=1) as wbig,
        tc.tile_pool(name="wconst", bufs=1) as wconst,
        tc.tile_pool(name="wsmall", bufs=1) as wsmall,
        tc.tile_pool(name="gat", bufs=2) as gpool,
        tc.tile_pool(name="work", bufs=2) as work,
        tc.tile_pool(name="chk", bufs=1) as chk,
        tc.tile_pool(name="sgrp", bufs=2) as sgrp,
        tc.tile_pool(name="ps_acc", bufs=1, space="PSUM") as ps_acc,
        tc.tile_pool(name="ps_tmp", bufs=2, space="PSUM") as ps_tmp,
        tc.tile_pool(name="ps_pool", bufs=1, space="PSUM") as ps_pool,
    ):
        # ---------- resident small tiles
        ident = wsmall.tile([128, 128], F32)
        nc.sync.dma_start(ident[:], wp["ident"][:])
        ones_col = wsmall.tile([128, 1], F32)
        nc.sync.dma_start(ones_col[:], wp["ones_col"][:])
        ones_lh = wsmall.tile([1, 128], F32)
        nc.sync.dma_start(ones_lh[:], wp["ones_lh"][:])
        sidx_sb = wsmall.tile([128, NCHUNK * ESLOT // 16], I16)
        nc.sync.dma_start(sidx_sb[:], src_idx[:])
        didx_sb = wsmall.tile([128, NCHUNK * ESLOT // 16], I16)
        nc.sync.dma_start(didx_sb[:], dst_idx[:])
        poh = wsmall.tile([128, NCHUNK, B], F32)
        nc.sync.dma_start(poh[:], pool_oh[:].rearrange("k p b -> p k b"))
        small = {}
        for name in ("gw1", "gb1", "gw2", "fw1", "fb1", "fw2", "fb2", "fw3", "fb3"):
            t = wsmall.tile(_WSHAPES[name], F32, tag=name)
            nc.sync.dma_start(t[:], wp[name][:])
            small[name] = t

        # ---------- layer-1 node transforms (K=49; full vl table every core)
        # xT_sb shares the big-weight slots (tag reuse) -- it dies before
        # the first wl_n/wr_n load.
        with tc.tile_pool(name="x1", bufs=1) as x1pool:
            xT_sb = wbig.tile([IN, N], F32, tag="wl")
            nc.sync.dma_start(xT_sb[:], xT[:])
            xTl_sb = x1pool.tile([IN, NLOC], F32)
            nc.sync.dma_start(xTl_sb[:], xTl[:])
            w1 = x1pool.tile([IN, 2 * D], F32)
            nc.sync.dma_start(w1[:, 0:D], wp["wl1"][:])
            nc.sync.dma_start(w1[:, D : 2 * D], wp["wr1"][:])
            b1 = x1pool.tile([1, 2 * D], F32)
            nc.sync.dma_start(b1[:, 0:D], wp["blb1"][:])
            nc.sync.dma_start(b1[:, D : 2 * D], wp["brb1"][:])

            ntiles = (N + 127) // 128
            for m in range(ntiles):
                lo = m * 128
                cnt = min(128, N - lo)
                sb = work.tile([128, D], F32, tag="wa")
                for half in range(2):
                    sl = slice(half * 512, (half + 1) * 512)
                    ps = ps_tmp.tile([128, 512], F32, tag="pt")
                    nc.tensor.matmul(
                        ps[0:cnt, :], xT_sb[:, lo : lo + cnt], w1[:, sl],
                        start=True, stop=False,
                    )
                    nc.tensor.matmul(
                        ps[0:cnt, :], ones_lh[:, 0:cnt], b1[:, sl],
                        start=False, stop=True,
                    )
                    nc.scalar.activation(sb[0:cnt, sl], ps[0:cnt, :], AF.Copy)
                nc.sync.dma_start(vl_tab[0][lo : lo + cnt, :], sb[0:cnt, :])
            for k in range(NCHUNK):
                lo = k * DCH
                sb = work.tile([128, D], F32, tag="wa")
                for half in range(2):
                    sl = slice(half * 512, (half + 1) * 512)
                    ps = ps_tmp.tile([128, 512], F32, tag="pt")
                    nc.tensor.matmul(
                        ps[0:DCH, :], xTl_sb[:, lo : lo + DCH],
                        w1[:, D : 2 * D][:, sl],
                        start=True, stop=False,
                    )
                    nc.tensor.matmul(
                        ps[0:DCH, :], ones_lh[:, 0:DCH], b1[:, D : 2 * D][:, sl],
                        start=False, stop=True,
                    )
                    nc.scalar.activation(sb[0:DCH, sl], ps[0:DCH, :], AF.Copy)
                nc.sync.dma_start(vr_tab[0][lo : lo + DCH, :], sb[0:DCH, :])

        # ---------- three GAT layers
        pool_num = ps_pool.tile([B, C], F32, tag="pnum")
        pool_den = ps_pool.tile([B, 8], F32, tag="pden")
        for li in range(3):
            att_bc = wconst.tile([128, D], F32, tag="att")
            nc.sync.dma_start(att_bc[:], wp[f"att_bc{li + 1}"][:])
            bnw = D if li < 2 else C
            bnsc = wconst.tile([128, bnw], F32, tag="bnsc")
            nc.sync.dma_start(bnsc[:], wp[f"bnsc{li + 1}"][:])
            bnsh = wconst.tile([128, bnw], F32, tag="bnsh")
            nc.sync.dma_start(bnsh[:], wp[f"bnsh{li + 1}"][:])
            if li < 2:
                wl_n = wbig.tile([128, H, D], F32, tag="wl")
                nc.sync.dma_start(
                    wl_n[:], wp[f"wl{li + 2}"][:].rearrange("(g p) n -> p g n", p=128)
                )
                wr_n = wbig.tile([128, H, D], F32, tag="wr")
                nc.sync.dma_start(
                    wr_n[:], wp[f"wr{li + 2}"][:].rearrange("(g p) n -> p g n", p=128)
                )
                b_n = wconst.tile([1, 2 * D], F32, tag="b_next")
                nc.sync.dma_start(b_n[:, 0:D], wp[f"blb{li + 2}"][:])
                nc.sync.dma_start(b_n[:, D : 2 * D], wp[f"brb{li + 2}"][:])

            for k in range(NCHUNK):
                num0 = ps_acc.tile([128, 512], F32, tag="num0")
                num1 = ps_acc.tile([128, 512], F32, tag="num1")
                den = ps_acc.tile([128, 8], F32, tag="den")
                for g in range(NG):
                    gl = gpool.tile([128, GKT, D], F32, tag="gl")
                    gr = gpool.tile([128, GKT, D], F32, tag="gr")
                    s_g = sgrp.tile([128, GKT, 128], F32, tag="s")
                    base = k * ESLOT + g * GKT * 128
                    nc.sync.dma_start(
                        s_g[:],
                        s_mat[base : base + GKT * 128, :].rearrange(
                            "(t p) j -> p t j", p=128
                        ),
                    )
                    ioff = base // 16
                    icnt = GKT * 128 // 16
                    nidx = GKT * 128
                    if "nogather" in BISECT:
                        nc.sync.dma_start(
                            gl[:], vl_tab[li][0 : GKT * 128, :].rearrange(
                                "(t p) d -> p t d", p=128))
                        nc.sync.dma_start(
                            gr[:], vr_tab[li][0 : GKT * 128, :].rearrange(
                                "(t p) d -> p t d", p=128))
                    else:
                        nc.gpsimd.dma_gather(
                            gl[:], vl_tab[li][:], sidx_sb[:, ioff : ioff + icnt],
                            nidx, nidx, D, single_packet=False,
                        )
                        nc.gpsimd.dma_gather(
                            gr[:], vr_tab[li][:], didx_sb[:, ioff : ioff + icnt],
                            nidx, nidx, D, single_packet=False,
                        )
                    for j in range(GKT):
                        t = g * GKT + j
                        vt = gl[:, j, :]
                        z = work.tile([128, D], F32, tag="wa")
                        nc.vector.tensor_tensor(z[:], vt, gr[:, j, :], ALU.add)
                        zl = work.tile([128, D], F32, tag="wb")
                        nc.vector.scalar_tensor_tensor(
                            zl[:], z[:], NEG, z[:], ALU.mult, ALU.max
                        )
                        e_t = work.tile([128, 8], F32, tag="e")
                        pfull = work.tile([128, D], F32, tag="wa")
                        nc.vector.tensor_tensor(pfull[:], zl[:], att_bc[:], ALU.mult)
                        nc.vector.tensor_reduce(
                            e_t[:],
                            pfull[:].rearrange("p (h c) -> p h c", h=H),
                            AX.X, ALU.add,
                        )
                        w_t = work.tile([128, 8], F32, tag="w")
                        nc.scalar.activation(w_t[:], e_t[:], AF.Exp)
                        g_t = work.tile([128, D], F32, tag="wa2")
                        for h in range(H):
                            nc.vector.tensor_scalar_mul(
                                g_t[:, h * C : (h + 1) * C],
                                vt[:, h * C : (h + 1) * C],
                                w_t[:, h : h + 1],
                            )
                        st = s_g[:, j, :]
                        nc.tensor.matmul(
                            num0[:], st, g_t[:, 0:512],
                            start=(t == 0), stop=(t == KT - 1),
                        )
                        nc.tensor.matmul(
                            num1[:], st, g_t[:, 512:1024],
                            start=(t == 0), stop=(t == KT - 1),
                        )
                        nc.tensor.matmul(
                            den[:], st, w_t[:],
                            start=(t == 0), stop=(t == KT - 1),
                        )
                # ---- chunk epilogue
                den_r = chk.tile([128, 8], F32, tag="denr")
                nc.vector.tensor_scalar_max(den_r[:], den[:], 1e-30)
                nc.vector.reciprocal(den_r[:], den_r[:])
                hpre = chk.tile([128, D], F32, tag="hpre")
                for h in range(H):
                    half = num0 if h < 4 else num1
                    sl = slice((h % 4) * C, (h % 4 + 1) * C)
                    nc.vector.tensor_scalar_mul(
                        hpre[:, h * C : (h + 1) * C], half[:, sl],
                        den_r[:, h : h + 1],
                    )
                if li < 2:
                    y_t = work.tile([128, D], F32, tag="wa")
                    nc.vector.tensor_tensor(y_t[:], hpre[:], bnsc[:], ALU.mult)
                    nc.vector.tensor_tensor(y_t[:], y_t[:], bnsh[:], ALU.add)
                    tmin = work.tile([128, D], F32, tag="wb")
                    nc.vector.tensor_scalar_min(tmin[:], y_t[:], 0.0)
                    expt = work.tile([128, D], F32, tag="wa2")
                    nc.scalar.activation(expt[:], tmin[:], AF.Exp)
                    hch = chk.tile([128, D], F32, tag="hch")
                    nc.vector.tensor_tensor(hch[:], y_t[:], tmin[:], ALU.subtract)
                    nc.vector.tensor_tensor(hch[:], hch[:], expt[:], ALU.add)
                    # transposes + next-layer transforms for this chunk
                    hT = chk.tile([128, H, DCH], F32, tag="hT")
                    for gi in range(H):
                        tps = ps_tmp.tile([128, 512], F32, tag="pt")
                        nc.tensor.transpose(
                            tps[:, 0:DCH],
                            hch[0:DCH, gi * C : (gi + 1) * C],
                            ident[0:DCH, 0:DCH],
                        )
                        nc.scalar.activation(hT[:, gi, :], tps[:, 0:DCH], AF.Copy)
                    vln = chk.tile([128, D], F32, tag="vln")
                    vrn = chk.tile([128, D], F32, tag="vrn")
                    for half in range(2):
                        sl = slice(half * 512, (half + 1) * 512)
                        psl = ps_tmp.tile([128, 512], F32, tag="pt")
                        psr = ps_tmp.tile([128, 512], F32, tag="pt")
                        for gi in range(H):
                            nc.tensor.matmul(
                                psl[0:DCH, :], hT[:, gi, :],
                                wl_n[:, gi, sl],
                                start=(gi == 0), stop=False,
                            )
                            nc.tensor.matmul(
                                psr[0:DCH, :], hT[:, gi, :],
                                wr_n[:, gi, sl],
                                start=(gi == 0), stop=False,
                            )
                        nc.tensor.matmul(
                            psl[0:DCH, :], ones_lh[:, 0:DCH], b_n[:, 0:D][:, sl],
                            start=False, stop=True,
                        )
                        nc.tensor.matmul(
                            psr[0:DCH, :], ones_lh[:, 0:DCH],
                            b_n[:, D : 2 * D][:, sl],
                            start=False, stop=True,
                        )
                        nc.scalar.activation(vln[0:DCH, sl], psl[0:DCH, :], AF.Copy)
                        nc.scalar.activation(vrn[0:DCH, sl], psr[0:DCH, :], AF.Copy)
                    lo = k * DCH
                    nc.sync.dma_start(
                        vl_bounce[li + 1][lo : lo + DCH, :], vln[0:DCH, :]
                    )
                    nc.sync.dma_start(
                        vr_tab[li + 1][lo : lo + DCH, :], vrn[0:DCH, :]
                    )
                else:
                    # layer 3: mean over heads (folded into bnsc) + BN + ELU'
                    hsum = chk.tile([128, C], F32, tag="hsum")
                    nc.vector.tensor_reduce(
                        hsum[:],
                        hpre[:].rearrange("p (h c) -> p c h", h=H),
                        AX.X, ALU.add,
                    )
                    y3 = chk.tile([128, C], F32, tag="y3")
                    nc.vector.tensor_tensor(y3[:], hsum[:], bnsc[:], ALU.mult)
                    nc.vector.tensor_tensor(y3[:], y3[:], bnsh[:], ALU.add)
                    t3 = work.tile([128, C], F32, tag="wb")
                    nc.vector.tensor_scalar_min(t3[:], y3[:], 0.0)
                    e3 = work.tile([128, C], F32, tag="wa2")
                    nc.scalar.activation(e3[:], t3[:], AF.Exp)
                    h3 = chk.tile([128, C], F32, tag="h3")
                    nc.vector.tensor_tensor(h3[:], y3[:], t3[:], ALU.subtract)
                    nc.vector.tensor_tensor(h3[:], h3[:], e3[:], ALU.add)
                    # gate
                    tps = ps_tmp.tile([128, 512], F32, tag="pt")
                    nc.tensor.transpose(
                        tps[:, 0:DCH], h3[0:DCH, :], ident[0:DCH, 0:DCH]
                    )
                    h3T = chk.tile([128, DCH], F32, tag="h3T")
                    nc.scalar.activation(h3T[:], tps[:, 0:DCH], AF.Copy)
                    g1ps = ps_tmp.tile([128, 512], F32, tag="pt")
                    nc.tensor.matmul(
                        g1ps[0:DCH, 0 : C // 2], h3T[:, 0:DCH], small["gw1"][:],
                        start=True, stop=False,
                    )
                    nc.tensor.matmul(
                        g1ps[0:DCH, 0 : C // 2], ones_lh[:, 0:DCH], small["gb1"][:],
                        start=False, stop=True,
                    )
                    g1 = chk.tile([128, C // 2], F32, tag="g1sb")
                    nc.scalar.activation(
                        g1[0:DCH, :], g1ps[0:DCH, 0 : C // 2], AF.Relu
                    )
                    tps2 = ps_tmp.tile([128, 512], F32, tag="pt")
                    nc.tensor.transpose(
                        tps2[0 : C // 2, 0:DCH], g1[0:DCH, :], ident[0:DCH, 0:DCH]
                    )
                    g1T = chk.tile([C // 2, DCH], F32, tag="g1T")
                    nc.scalar.activation(g1T[:], tps2[0 : C // 2, 0:DCH], AF.Copy)
                    gps = ps_tmp.tile([128, 512], F32, tag="pt")
                    nc.tensor.matmul(
                        gps[0:DCH, 0:1], g1T[:, 0:DCH], small["gw2"][:],
                        start=True, stop=True,
                    )
                    expg = chk.tile([128, 1], F32, tag="expg")
                    nc.scalar.activation(expg[0:DCH, :], gps[0:DCH, 0:1], AF.Exp)
                    p_t = chk.tile([128, B], F32, tag="poolP")
                    nc.vector.tensor_scalar_mul(
                        p_t[0:DCH, :], poh[0:DCH, k, :], expg[0:DCH, :]
                    )
                    nc.tensor.matmul(
                        pool_num[:], p_t[0:DCH, :], h3[0:DCH, :],
                        start=(k == 0), stop=(k == NCHUNK - 1),
                    )
                    nc.tensor.matmul(
                        pool_den[:, 0:1], p_t[0:DCH, :], ones_col[0:DCH, :],
                        start=(k == 0), stop=(k == NCHUNK - 1),
                    )
            if li < 2:
                if "nocc" in BISECT:
                    for r in range(NCORES):
                        nc.sync.dma_start(
                            vl_shared[li + 1][r * NLOC : (r + 1) * NLOC, :],
                            vl_bounce[li + 1][:],
                        )
                else:
                    nc.gpsimd.collective_compute(
                        "AllGather",
                        ALU.bypass,
                        replica_groups=[list(range(NCORES))],
                        ins=[vl_bounce[li + 1].ap().opt()],
                        outs=[vl_shared[li + 1].ap().opt()],
                    )

        # ---------- cross-core pooling reduce + MLP head
        psb = chk.tile([B, C + 1], F32, tag="psb")
        nc.scalar.activation(psb[:, 0:C], pool_num[:], AF.Copy)
        nc.scalar.activation(psb[:, C : C + 1], pool_den[:, 0:1], AF.Copy)
        nc.sync.dma_start(pool_bounce[:], psb[:])
        if "nocc" in BISECT:
            nc.sync.dma_start(pool_shared[:], pool_bounce[:])
        else:
            nc.gpsimd.collective_compute(
                "AllReduce",
                ALU.add,
                replica_groups=[list(range(NCORES))],
                ins=[pool_bounce.ap().opt()],
                outs=[pool_shared.ap().opt()],
            )
        pall = chk.tile([B, C + 1], F32, tag="pall")
        nc.sync.dma_start(pall[:], pool_shared[:])
        denp = chk.tile([B, 1], F32, tag="denp")
        nc.vector.reciprocal(denp[:], pall[:, C : C + 1])
        pooled = chk.tile([B, C], F32, tag="pooled")
        nc.vector.tensor_scalar_mul(pooled[:], pall[:, 0:C], denp[:])

        def head_mm(inp, w_t, b_t, act, tag):
            kdim, ncols = w_t.shape[0], w_t.shape[1]
            tp = ps_tmp.tile([128, 512], F32, tag="pt")
            nc.tensor.transpose(tp[0:kdim, 0:B], inp, ident[0:B, 0:B])
            tsb = chk.tile([128, B], F32, tag=f"{tag}t")
            nc.scalar.activation(tsb[0:kdim, :], tp[0:kdim, 0:B], AF.Copy)
            ops = ps_tmp.tile([128, 512], F32, tag="pt")
            nc.tensor.matmul(
                ops[0:B, 0:ncols], tsb[0:kdim, 0:B], w_t[:], start=True, stop=False
            )
            nc.tensor.matmul(
                ops[0:B, 0:ncols], ones_lh[:, 0:B], b_t[:], start=False, stop=True
            )
            osb = chk.tile([B, max(ncols, 1)], F32, tag=f"{tag}o")
            nc.scalar.activation(osb[:, 0:ncols], ops[0:B, 0:ncols], act)
            return osb

        o1 = head_mm(pooled[:], small["fw1"], small["fb1"], AF.Relu, "o1")
        o2 = head_mm(o1[:], small["fw2"], small["fb2"], AF.Relu, "o2")
        o3 = head_mm(o2[:], small["fw3"], small["fb3"], AF.Copy, "o3")
        nc.sync.dma_start(out_y[:], o3[:, 0:1])


# ---------------------------------------------------------------- public API
def _in_maps(x, edge_index, batch, params):
    x = np.asarray(x, np.float32)
    graph = _prep_graph(np.asarray(edge_index))
    pools = _prep_pool(np.asarray(batch))
    p = _prep_params(params)
    xt = np.ascontiguousarray(x.T)
    maps = []
    for c in range(NCORES):
        m = dict(
            xT=xt,
            xTl=np.ascontiguousarray(xt[:, c * NLOC : (c + 1) * NLOC]),
            src_idx=graph[c]["src_idx"],
            dst_idx=graph[c]["dst_idx"],
            s_mat=graph[c]["s_mat"],
            pool_oh=pools[c],
        )
        for name in _WSHAPES:
            m[name] = np.ascontiguousarray(p[name], np.float32)
        maps.append(m)
    return maps


def get_nc():
    if "nc" not in _CACHE:
        _CACHE["nc"] = _build()
    return _CACHE["nc"]


def kernel(x, edge_index, batch, params):
    nc = get_nc()
    res = run_bass_kernel_spmd(
        nc, _in_maps(x, edge_index, batch, params), list(range(NCORES))
    )
    return np.asarray(res.results[0]["y"], np.float32)


# revision 19
# speedup vs baseline: 1.6187x; 1.6187x over previous
"""Trainium2 Bass kernel for nn_BindingAffinityGAT (3x GATv2 + BN + ELU,
global attention pooling, MLP head) on 8 NeuronCores.

Sharding: nodes partitioned by dst across the 8 cores (1250 each); edges
live with their destination core (fixed 125-dst chunks x 18 K-tiles of 128
edge slots, zero-padded so all cores run one SPMD program); per-layer halo
exchange (AllGather) of the source-side transformed features; weights
replicated. Aggregation + segment softmax run as PE matmuls against a
host-built edge->dst one-hot (S) so variable-degree segments never touch
the vector engine.

Self-contained: hardcodes all shapes; host-side numpy does graph prep.
"""

import sys
import types

import numpy as np
import ml_dtypes


def _install_ntff_hook():
    if "antenv.axon_hooks" in sys.modules:
        return
    mod = types.ModuleType("antenv.axon_hooks")
    mod._hook = None
    mod.set_axon_ntff_profile_hook = lambda h: setattr(mod, "_hook", h)
    mod.get_axon_ntff_profile_hook = lambda: mod._hook
    sys.modules["antenv.axon_hooks"] = mod
    import antenv

    antenv.axon_hooks = mod
    try:
        from trn_agent_boot.trn_boot import _ntff_profile_via_ctypes

        mod._hook = _ntff_profile_via_ctypes("/opt/axon/libaxon_pjrt.so")
    except Exception:
        pass


_install_ntff_hook()

import concourse.bacc as bacc
import concourse.mybir as mybir
import concourse.tile as tile
from concourse import library_config
from concourse.bass_utils import run_bass_kernel_spmd

# ---------------------------------------------------------------- constants
N = 10000
E = 160000
IN = 49
H = 8
C = 128
D = H * C  # 1024
B = 16
NEG = 0.2
EPS = 1e-5

NCORES = 8
NLOC = N // NCORES  # 1250
NCHUNK = 10  # dst chunks per core
DCH = NLOC // NCHUNK  # 125 dsts per chunk
KT = 18  # K-tiles (128 edge slots) per chunk; fixed across cores
ESLOT = KT * 128  # 2304 edge slots per chunk
GKT = 3  # K-tiles per gather group
NG = KT // GKT  # gather groups per chunk
F32 = mybir.dt.float32
BF16 = mybir.dt.bfloat16
I16 = mybir.dt.int16
AX = mybir.AxisListType
ALU = mybir.AluOpType
AF = mybir.ActivationFunctionType

_CACHE: dict = {}
BF_PARAMS = {"onesb_lh", "xT", "xTl", "s_mat", "wl1", "wr1", "wl2", "wr2", "wl3", "wr3",
             "blb1", "brb1", "blb2", "brb2", "blb3", "brb3", "att_bc1", "att_bc2",
             "att_bc3", "identb"}
import os  # noqa: E402
BISECT = os.environ.get('KBISECT', '')


# ---------------------------------------------------------------- host prep
def _wrap_idx(lin):
    assert len(lin) % 16 == 0
    w = lin.reshape(-1, 16).T  # [16, S]; linear j -> (j % 16, j // 16)
    return np.tile(w, (8, 1)).astype(np.int16)


def _prep_graph(edge_index):
    src = np.concatenate([edge_index[0], np.arange(N)]).astype(np.int64)
    dst = np.concatenate([edge_index[1], np.arange(N)]).astype(np.int64)
    order = np.argsort(dst, kind="stable")
    src, dst = src[order], dst[order]

    cores = []
    for c in range(NCORES):
        lo, hi = c * NLOC, (c + 1) * NLOC
        m = (dst >= lo) & (dst < hi)
        cs, cd = src[m], dst[m] - lo
        src_slots = np.zeros(NCHUNK * ESLOT, np.int64)
        dst_slots = np.zeros(NCHUNK * ESLOT, np.int64)
        s_mat = np.zeros((NCHUNK * ESLOT, 128), np.float32)
        for k in range(NCHUNK):
            dlo, dhi = k * DCH, (k + 1) * DCH
            km = (cd >= dlo) & (cd < dhi)
            ks, kd = cs[km], cd[km] - dlo
            ne = len(ks)
            assert ne <= ESLOT, f"chunk overflow: {ne} > {ESLOT}"
            base = k * ESLOT
            src_slots[base : base + ne] = ks
            dst_slots[base : base + ne] = kd + dlo
            s_mat[np.arange(base, base + ne), kd] = 1.0
        cores.append(
            dict(
                src_idx=_wrap_idx(src_slots.astype(np.int16)),
                dst_idx=_wrap_idx(dst_slots.astype(np.int16)),
                s_mat=s_mat,
            )
        )
    return cores


def _prep_pool(batch):
    cores = []
    for c in range(NCORES):
        lo = c * NLOC
        tiles = []
        for k in range(NCHUNK):
            oh = np.zeros((128, B), np.float32)
            seg = np.asarray(batch)[lo + k * DCH : lo + (k + 1) * DCH]
            oh[np.arange(DCH), seg] = 1.0
            tiles.append(oh)
        cores.append(np.stack(tiles).astype(np.float32))
    return cores


def _rep128(v):
    return np.tile(np.asarray(v, np.float32).reshape(1, -1), (128, 1))


def _prep_params(params):
    p = {}

    def bn_affine(bn):
        sc = np.asarray(bn["gamma"] / np.sqrt(bn["var"] + EPS), np.float32)
        sh = np.asarray(bn["beta"] - bn["mean"] * sc, np.float32)
        return sc, sh

    for li, gk, bk in ((1, "gat1", "bn1"), (2, "gat2", "bn2"), (3, "gat3", "bn3")):
        g = params[gk]
        p[f"wl{li}"] = np.asarray(g["Wl"], np.float32)
        p[f"wr{li}"] = np.asarray(g["Wr"], np.float32)
        p[f"blb{li}"] = np.asarray(g["bl"], np.float32).reshape(1, D)
        p[f"brb{li}"] = np.asarray(g["br"], np.float32).reshape(1, D)
        att = np.asarray(g["att"], np.float32)
        p[f"att_bc{li}"] = _rep128(att.reshape(D))
        sc, sh = bn_affine(params[bk])
        bias = np.asarray(g["bias"], np.float32)
        if li < 3:
            p[f"bnsc{li}"] = _rep128(sc)
            p[f"bnsh{li}"] = _rep128(sh + bias * sc)
        else:
            p[f"bnsc{li}"] = _rep128(sc / H)  # fold mean-over-heads
            p[f"bnsh{li}"] = _rep128(sh + bias * sc)
    # device h := elu+1 (>=0); fold the -1 into consumers' biases
    for li in (2, 3):
        for side in ("l", "r"):
            w = p[f"w{side}{li}"]
            p[f"b{side}b{li}"] = p[f"b{side}b{li}"] - w.sum(0).reshape(1, D)
    gw1 = np.asarray(params["gate_w1"], np.float32)
    gb1 = np.asarray(params["gate_b1"], np.float32).reshape(1, -1) - gw1.sum(0)
    gw2 = np.asarray(params["gate_w2"], np.float32)
    fw1 = np.asarray(params["fc1_w"], np.float32)
    fb1 = np.asarray(params["fc1_b"], np.float32).reshape(1, -1) - fw1.sum(0)
    p.update(
        gw1=gw1, gb1=gb1, gw2=gw2, fw1=fw1, fb1=fb1,
        fw2=np.asarray(params["fc2_w"], np.float32),
        fb2=np.asarray(params["fc2_b"], np.float32).reshape(1, -1),
        fw3=np.asarray(params["fc3_w"], np.float32),
        fb3=np.asarray(params["fc3_b"], np.float32).reshape(1, -1),
        ident=np.eye(128, dtype=np.float32),
        identb=np.eye(128, dtype=np.float32),
        ones_col=np.ones((128, 1), np.float32),
        ones_lh=np.ones((1, 128), np.float32),
        onesb_lh=np.ones((1, 128), np.float32),
    )
    return p


# ---------------------------------------------------------------- device build
_WSHAPES = {
    "wl1": [IN, D], "wr1": [IN, D], "blb1": [1, D], "brb1": [1, D],
    "wl2": [D, D], "wr2": [D, D], "blb2": [1, D], "brb2": [1, D],
    "wl3": [D, D], "wr3": [D, D], "blb3": [1, D], "brb3": [1, D],
    "att_bc1": [128, D], "bnsc1": [128, D], "bnsh1": [128, D],
    "att_bc2": [128, D], "bnsc2": [128, D], "bnsh2": [128, D],
    "att_bc3": [128, D], "bnsc3": [128, C], "bnsh3": [128, C],
    "gw1": [C, C // 2], "gb1": [1, C // 2], "gw2": [C // 2, 1],
    "fw1": [C, 128], "fb1": [1, 128], "fw2": [128, 64], "fb2": [1, 64],
    "fw3": [64, 1], "fb3": [1, 1], "ident": [128, 128],
    "ones_col": [128, 1], "ones_lh": [1, 128], "identb": [128, 128],
    "xT": [IN, N], "xTl": [IN, NLOC], "s_mat": [NCHUNK * ESLOT, 128],
    "onesb_lh": [1, 128],
}


def _build():
    nc = bacc.Bacc("TRN2", target_bir_lowering=False, debug=False, num_devices=NCORES)

    src_idx = nc.declare_dram_parameter(
        "src_idx", [128, NCHUNK * ESLOT // 16], I16, isOutput=False
    )
    dst_idx = nc.declare_dram_parameter(
        "dst_idx", [128, NCHUNK * ESLOT // 16], I16, isOutput=False
    )
    pool_oh = nc.declare_dram_parameter("pool_oh", [NCHUNK, 128, B], F32, isOutput=False)
    wp = {
        name: nc.declare_dram_parameter(
            name, shape, BF16 if name in BF_PARAMS else F32, isOutput=False
        )
        for name, shape in _WSHAPES.items()
    }
    out_y = nc.declare_dram_parameter("y", [B, 1], F32, isOutput=True)

    vl_tab1 = nc.dram_tensor("vl_tab1", [N, D], BF16)
    vr_tab = [nc.dram_tensor(f"vr_tab{li}", [NLOC, D], BF16) for li in (1, 2, 3)]
    vl_bounce = [None,
                 nc.dram_tensor("vl_b2", [NLOC, D], BF16),
                 nc.dram_tensor("vl_b3", [NLOC, D], BF16)]
    vl_shared = [None,
                 nc.dram_tensor("vl_s2", [N, D], BF16, addr_space="Shared"),
                 nc.dram_tensor("vl_s3", [N, D], BF16, addr_space="Shared")]
    vl_tab = [vl_tab1, vl_shared[1], vl_shared[2]]
    pool_bounce = nc.dram_tensor("pool_bounce", [B, C + 1], F32)
    pool_shared = nc.dram_tensor("pool_shared", [B, C + 1], F32, addr_space="Shared")

    xT = wp["xT"]
    xTl = wp["xTl"]
    s_mat = wp["s_mat"]
    with tile.TileContext(nc) as tc:
        nc.gpsimd.load_library(library_config.mlp)
        _emit(nc, tc, xT, xTl, src_idx, dst_idx, s_mat, pool_oh, wp,
              vl_tab, vr_tab, vl_bounce, vl_shared, pool_bounce, pool_shared,
              out_y)
    nc.compile()
    return nc


def _emit(nc, tc, xT, xTl, src_idx, dst_idx, s_mat, pool_oh, wp,
          vl_tab, vr_tab, vl_bounce, vl_shared, pool_bounce, pool_shared,
          out_y):
    with (
        tc.tile_pool(name="wbig", bufs=1) as wbig,
        tc.tile_pool(name="wconst", bufs=1) as wconst,
        tc.tile_pool(name="wsmall", bufs=1) as wsmall,
        tc.tile_pool(name="gat", bufs=2) as gpool,
        tc.tile_pool(name="work", bufs=2) as work,
        tc.tile_pool(name="chk", bufs=1) as chk,
        tc.tile_pool(name="sgrp", bufs=2) as sgrp,
        tc.tile_pool(name="ps_acc", bufs=1, space="PSUM") as ps_acc,
        tc.tile_pool(name="ps_tmp", bufs=2, space="PSUM") as ps_tmp,
        tc.tile_pool(name="ps_pool", bufs=1, space="PSUM") as ps_pool,
    ):
        # ---------- resident small tiles
        ident = wsmall.tile([128, 128], F32)
        nc.sync.dma_start(ident[:], wp["ident"][:])
        identb = wsmall.tile([128, 128], BF16, tag="identb")
        nc.sync.dma_start(identb[:], wp["identb"][:])
        onesb_lh = wsmall.tile([1, 128], BF16, tag="onesb")
        nc.sync.dma_start(onesb_lh[:], wp["onesb_lh"][:])
        ones_col = wsmall.tile([128, 1], F32)
        nc.sync.dma_start(ones_col[:], wp["ones_col"][:])
        ones_lh = wsmall.tile([1, 128], F32)
        nc.sync.dma_start(ones_lh[:], wp["ones_lh"][:])
        sidx_sb = wsmall.tile([128, NCHUNK * ESLOT // 16], I16)
        nc.sync.dma_start(sidx_sb[:], src_idx[:])
        didx_sb = wsmall.tile([128, NCHUNK * ESLOT // 16], I16)
        nc.sync.dma_start(didx_sb[:], dst_idx[:])
        poh = wsmall.tile([128, NCHUNK, B], F32)
        nc.sync.dma_start(poh[:], pool_oh[:].rearrange("k p b -> p k b"))
        small = {}
        for name in ("gw1", "gb1", "gw2", "fw1", "fb1", "fw2", "fb2", "fw3", "fb3"):
            t = wsmall.tile(_WSHAPES[name], F32, tag=name)
            nc.sync.dma_start(t[:], wp[name][:])
            small[name] = t

        # ---------- layer-1 node transforms (K=49; full vl table every core)
        # xT_sb shares the big-weight slots (tag reuse) -- it dies before
        # the first wl_n/wr_n load.
        with tc.tile_pool(name="x1", bufs=1) as x1pool:
            xT_sb = wbig.tile([IN, N], BF16, tag="wl")
            nc.sync.dma_start(xT_sb[:], xT[:])
            xTl_sb = x1pool.tile([IN, NLOC], BF16)
            nc.sync.dma_start(xTl_sb[:], xTl[:])
            w1 = x1pool.tile([IN, 2 * D], BF16)
            nc.sync.dma_start(w1[:, 0:D], wp["wl1"][:])
            nc.sync.dma_start(w1[:, D : 2 * D], wp["wr1"][:])
            b1 = x1pool.tile([1, 2 * D], BF16)
            nc.sync.dma_start(b1[:, 0:D], wp["blb1"][:])
            nc.sync.dma_start(b1[:, D : 2 * D], wp["brb1"][:])

            ntiles = (N + 127) // 128
            for m in range(ntiles):
                lo = m * 128
                cnt = min(128, N - lo)
                sb = work.tile([128, D], BF16, tag="wab")
                for half in range(2):
                    sl = slice(half * 512, (half + 1) * 512)
                    ps = ps_tmp.tile([128, 512], F32, tag="pt")
                    nc.tensor.matmul(
                        ps[0:cnt, :], xT_sb[:, lo : lo + cnt], w1[:, sl],
                        start=True, stop=False,
                    )
                    nc.tensor.matmul(
                        ps[0:cnt, :], onesb_lh[:, 0:cnt], b1[:, sl],
                        start=False, stop=True,
                    )
                    nc.scalar.activation(sb[0:cnt, sl], ps[0:cnt, :], AF.Copy)
                nc.sync.dma_start(vl_tab[0][lo : lo + cnt, :], sb[0:cnt, :])
            for k in range(NCHUNK):
                lo = k * DCH
                sb = work.tile([128, D], BF16, tag="wab")
                for half in range(2):
                    sl = slice(half * 512, (half + 1) * 512)
                    ps = ps_tmp.tile([128, 512], F32, tag="pt")
                    nc.tensor.matmul(
                        ps[0:DCH, :], xTl_sb[:, lo : lo + DCH],
                        w1[:, D : 2 * D][:, sl],
                        start=True, stop=False,
                    )
                    nc.tensor.matmul(
                        ps[0:DCH, :], onesb_lh[:, 0:DCH], b1[:, D : 2 * D][:, sl],
                        start=False, stop=True,
                    )
                    nc.scalar.activation(sb[0:DCH, sl], ps[0:DCH, :], AF.Copy)
                nc.sync.dma_start(vr_tab[0][lo : lo + DCH, :], sb[0:DCH, :])

        # ---------- three GAT layers
        pool_num = ps_pool.tile([B, C], F32, tag="pnum")
        pool_den = ps_pool.tile([B, 8], F32, tag="pden")
        for li in range(3):
            att_bc = wconst.tile([128, D], BF16, tag="att")
            nc.sync.dma_start(att_bc[:], wp[f"att_bc{li + 1}"][:])
            bnw = D if li < 2 else C
            bnsc = wconst.tile([128, bnw], F32, tag="bnsc")
            nc.sync.dma_start(bnsc[:], wp[f"bnsc{li + 1}"][:])
            bnsh = wconst.tile([128, bnw], F32, tag="bnsh")
            nc.sync.dma_start(bnsh[:], wp[f"bnsh{li + 1}"][:])
            if li < 2:
                wl_n = wbig.tile([128, H, D], BF16, tag="wl")
                nc.sync.dma_start(
                    wl_n[:], wp[f"wl{li + 2}"][:].rearrange("(g p) n -> p g n", p=128)
                )
                wr_n = wbig.tile([128, H, D], BF16, tag="wr")
                nc.sync.dma_start(
                    wr_n[:], wp[f"wr{li + 2}"][:].rearrange("(g p) n -> p g n", p=128)
                )
                b_n = wconst.tile([1, 2 * D], BF16, tag="b_next")
                nc.sync.dma_start(b_n[:, 0:D], wp[f"blb{li + 2}"][:])
                nc.sync.dma_start(b_n[:, D : 2 * D], wp[f"brb{li + 2}"][:])

            for k in range(NCHUNK):
                num0 = ps_acc.tile([128, 512], F32, tag="num0")
                num1 = ps_acc.tile([128, 512], F32, tag="num1")
                den = ps_acc.tile([128, 8], F32, tag="den")
                for g in range(NG):
                    gl = gpool.tile([128, GKT, D], BF16, tag="gl")
                    gr = gpool.tile([128, GKT, D], BF16, tag="gr")
                    s_g = sgrp.tile([128, GKT, 128], BF16, tag="s")
                    base = k * ESLOT + g * GKT * 128
                    nc.sync.dma_start(
                        s_g[:],
                        s_mat[base : base + GKT * 128, :].rearrange(
                            "(t p) j -> p t j", p=128
                        ),
                    )
                    ioff = base // 16
                    icnt = GKT * 128 // 16
                    nidx = GKT * 128
                    if "nogather" in BISECT:
                        nc.sync.dma_start(
                            gl[:], vl_tab[li][0 : GKT * 128, :].rearrange(
                                "(t p) d -> p t d", p=128))
                        nc.sync.dma_start(
                            gr[:], vr_tab[li][0 : GKT * 128, :].rearrange(
                                "(t p) d -> p t d", p=128))
                    else:
                        nc.gpsimd.dma_gather(
                            gl[:], vl_tab[li][:], sidx_sb[:, ioff : ioff + icnt],
                            nidx, nidx, D, single_packet=False,
                        )
                        nc.gpsimd.dma_gather(
                            gr[:], vr_tab[li][:], didx_sb[:, ioff : ioff + icnt],
                            nidx, nidx, D, single_packet=False,
                        )
                    for j in range(GKT):
                        t = g * GKT + j
                        vt = gl[:, j, :]
                        z = work.tile([128, D], BF16, tag="wa")
                        nc.vector.tensor_tensor(z[:], vt, gr[:, j, :], ALU.add)
                        zl = work.tile([128, D], BF16, tag="wb")
                        nc.vector.scalar_tensor_tensor(
                            zl[:], z[:], NEG, z[:], ALU.mult, ALU.max
                        )
                        e_t = work.tile([128, 8], F32, tag="e")
                        pfull = work.tile([128, D], BF16, tag="wa")
                        nc.vector.tensor_tensor(pfull[:], zl[:], att_bc[:], ALU.mult)
                        nc.vector.tensor_reduce(
                            e_t[:],
                            pfull[:].rearrange("p (h c) -> p h c", h=H),
                            AX.X, ALU.add,
                        )
                        w_t = work.tile([128, 8], F32, tag="w")
                        nc.scalar.activation(w_t[:], e_t[:], AF.Exp)
                        w_b = work.tile([128, 8], BF16, tag="wbh")
                        nc.vector.tensor_copy(w_b[:], w_t[:])
                        g_t = work.tile([128, D], BF16, tag="wa2")
                        for h in range(H):
                            nc.vector.tensor_scalar_mul(
                                g_t[:, h * C : (h + 1) * C],
                                vt[:, h * C : (h + 1) * C],
                                w_t[:, h : h + 1],
                            )
                        st = s_g[:, j, :]
                        nc.tensor.matmul(
                            num0[:], st, g_t[:, 0:512],
                            start=(t == 0), stop=(t == KT - 1),
                        )
                        nc.tensor.matmul(
                            num1[:], st, g_t[:, 512:1024],
                            start=(t == 0), stop=(t == KT - 1),
                        )
                        nc.tensor.matmul(
                            den[:], st, w_b[:],
                            start=(t == 0), stop=(t == KT - 1),
                        )
                # ---- chunk epilogue
                den_r = chk.tile([128, 8], F32, tag="denr")
                nc.vector.tensor_scalar_max(den_r[:], den[:], 1e-30)
                nc.vector.reciprocal(den_r[:], den_r[:])
                hpre = chk.tile([128, D], F32, tag="hpre")
                for h in range(H):
                    half = num0 if h < 4 else num1
                    sl = slice((h % 4) * C, (h % 4 + 1) * C)
                    nc.vector.tensor_scalar_mul(
                        hpre[:, h * C : (h + 1) * C], half[:, sl],
                        den_r[:, h : h + 1],
                    )
                if li < 2:
                    y_t = work.tile([128, D], F32, tag="wa")
                    nc.vector.tensor_tensor(y_t[:], hpre[:], bnsc[:], ALU.mult)
                    nc.vector.tensor_tensor(y_t[:], y_t[:], bnsh[:], ALU.add)
                    tmin = work.tile([128, D], F32, tag="wb")
                    nc.vector.tensor_scalar_min(tmin[:], y_t[:], 0.0)
                    expt = work.tile([128, D], F32, tag="wa2")
                    nc.scalar.activation(expt[:], tmin[:], AF.Exp)
                    hch = chk.tile([128, D], BF16, tag="hch")
                    nc.vector.tensor_tensor(hch[:], y_t[:], tmin[:], ALU.subtract)
                    nc.vector.tensor_tensor(hch[:], hch[:], expt[:], ALU.add)
                    # transposes + next-layer transforms for this chunk
                    hT = chk.tile([128, H, DCH], BF16, tag="hT")
                    for gi in range(H):
                        tps = ps_tmp.tile([128, 512], BF16, tag="pt")
                        nc.tensor.transpose(
                            tps[:, 0:DCH],
                            hch[0:DCH, gi * C : (gi + 1) * C],
                            identb[0:DCH, 0:DCH],
                        )
                        nc.scalar.activation(hT[:, gi, :], tps[:, 0:DCH], AF.Copy)
                    vln = chk.tile([128, D], BF16, tag="vln")
                    vrn = chk.tile([128, D], BF16, tag="vrn")
                    for half in range(2):
                        sl = slice(half * 512, (half + 1) * 512)
                        psl = ps_tmp.tile([128, 512], F32, tag="pt")
                        psr = ps_tmp.tile([128, 512], F32, tag="pt")
                        for gi in range(H):
                            nc.tensor.matmul(
                                psl[0:DCH, :], hT[:, gi, :],
                                wl_n[:, gi, sl],
                                start=(gi == 0), stop=False,
                            )
                            nc.tensor.matmul(
                                psr[0:DCH, :], hT[:, gi, :],
                                wr_n[:, gi, sl],
                                start=(gi == 0), stop=False,
                            )
                        nc.tensor.matmul(
                            psl[0:DCH, :], onesb_lh[:, 0:DCH], b_n[:, 0:D][:, sl],
                            start=False, stop=True,
                        )
                        nc.tensor.matmul(
                            psr[0:DCH, :], onesb_lh[:, 0:DCH],
                            b_n[:, D : 2 * D][:, sl],
                            start=False, stop=True,
                        )
                        nc.scalar.activation(vln[0:DCH, sl], psl[0:DCH, :], AF.Copy)
                        nc.scalar.activation(vrn[0:DCH, sl], psr[0:DCH, :], AF.Copy)
                    lo = k * DCH
                    nc.sync.dma_start(
                        vl_bounce[li + 1][lo : lo + DCH, :], vln[0:DCH, :]
                    )
                    nc.sync.dma_start(
                        vr_tab[li + 1][lo : lo + DCH, :], vrn[0:DCH, :]
                    )
                else:
                    # layer 3: mean over heads (folded into bnsc) + BN + ELU'
                    hsum = chk.tile([128, C], F32, tag="hsum")
                    nc.vector.tensor_reduce(
                        hsum[:],
                        hpre[:].rearrange("p (h c) -> p c h", h=H),
                        AX.X, ALU.add,
                    )
                    y3 = chk.tile([128, C], F32, tag="y3")
                    nc.vector.tensor_tensor(y3[:], hsum[:], bnsc[:], ALU.mult)
                    nc.vector.tensor_tensor(y3[:], y3[:], bnsh[:], ALU.add)
                    t3 = work.tile([128, C], F32, tag="wb")
                    nc.vector.tensor_scalar_min(t3[:], y3[:], 0.0)
                    e3 = work.tile([128, C], F32, tag="wa2")
                    nc.scalar.activation(e3[:], t3[:], AF.Exp)
                    h3 = chk.tile([128, C], F32, tag="h3")
                    nc.vector.tensor_tensor(h3[:], y3[:], t3[:], ALU.subtract)
                    nc.vector.tensor_tensor(h3[:], h3[:], e3[:], ALU.add)
                    # gate
                    tps = ps_tmp.tile([128, 512], F32, tag="pt")
                    nc.tensor.transpose(
                        tps[:, 0:DCH], h3[0:DCH, :], ident[0:DCH, 0:DCH]
                    )
                    h3T = chk.tile([128, DCH], F32, tag="h3T")
                    nc.scalar.activation(h3T[:], tps[:, 0:DCH], AF.Copy)
                    g1ps = ps_tmp.tile([128, 512], F32, tag="pt")
                    nc.tensor.matmul(
                        g1ps[0:DCH, 0 : C // 2], h3T[:, 0:DCH], small["gw1"][:],
                        start=True, stop=False,
                    )
                    nc.tensor.matmul(
                        g1ps[0:DCH, 0 : C // 2], ones_lh[:, 0:DCH], small["gb1"][:],
                        start=False, stop=True,
                    )
                    g1 = chk.tile([128, C // 2], F32, tag="g1sb")
                    nc.scalar.activation(
                        g1[0:DCH, :], g1ps[0:DCH, 0 : C // 2], AF.Relu
                    )
                    tps2 = ps_tmp.tile([128, 512], F32, tag="pt")
                    nc.tensor.transpose(
                        tps2[0 : C // 2, 0:DCH], g1[0:DCH, :], ident[0:DCH, 0:DCH]
                    )
                    g1T = chk.tile([C // 2, DCH], F32, tag="g1T")
                    nc.scalar.activation(g1T[:], tps2[0 : C // 2, 0:DCH], AF.Copy)
                    gps = ps_tmp.tile([128, 512], F32, tag="pt")
                    nc.tensor.matmul(
                        gps[0:DCH, 0:1], g1T[:, 0:DCH], small["gw2"][:],
                        start=True, stop=True,
                    )
                    expg = chk.tile([128, 1], F32, tag="expg")
                    nc.scalar.activation(expg[0:DCH, :], gps[0:DCH, 0:1], AF.Exp)
                    p_t = chk.tile([128, B], F32, tag="poolP")
                    nc.vector.tensor_scalar_mul(
                        p_t[0:DCH, :], poh[0:DCH, k, :], expg[0:DCH, :]
                    )
                    nc.tensor.matmul(
                        pool_num[:], p_t[0:DCH, :], h3[0:DCH, :],
                        start=(k == 0), stop=(k == NCHUNK - 1),
                    )
                    nc.tensor.matmul(
                        pool_den[:, 0:1], p_t[0:DCH, :], ones_col[0:DCH, :],
                        start=(k == 0), stop=(k == NCHUNK - 1),
                    )
            if li < 2:
                if "nocc" in BISECT:
                    for r in range(NCORES):
                        nc.sync.dma_start(
                            vl_shared[li + 1][r * NLOC : (r + 1) * NLOC, :],
                            vl_bounce[li + 1][:],
                        )
                else:
                    nc.gpsimd.collective_compute(
                        "AllGather",
                        ALU.bypass,
                        replica_groups=[list(range(NCORES))],
                        ins=[vl_bounce[li + 1].ap().opt()],
                        outs=[vl_shared[li + 1].ap().opt()],
                    )

        # ---------- cross-core pooling reduce + MLP head
        psb = chk.tile([B, C + 1], F32, tag="psb")
        nc.scalar.activation(psb[:, 0:C], pool_num[:], AF.Copy)
        nc.scalar.activation(psb[:, C : C + 1], pool_den[:, 0:1], AF.Copy)
        nc.sync.dma_start(pool_bounce[:], psb[:])
        if "nocc" in BISECT:
            nc.sync.dma_start(pool_shared[:], pool_bounce[:])
        else:
            nc.gpsimd.collective_compute(
                "AllReduce",
                ALU.add,
                replica_groups=[list(range(NCORES))],
                ins=[pool_bounce.ap().opt()],
                outs=[pool_shared.ap().opt()],
            )
        pall = chk.tile([B, C + 1], F32, tag="pall")
        nc.sync.dma_start(pall[:], pool_shared[:])
        denp = chk.tile([B, 1], F32, tag="denp")
        nc.vector.reciprocal(denp[:], pall[:, C : C + 1])
        pooled = chk.tile([B, C], F32, tag="pooled")
        nc.vector.tensor_scalar_mul(pooled[:], pall[:, 0:C], denp[:])

        def head_mm(inp, w_t, b_t, act, tag):
            kdim, ncols = w_t.shape[0], w_t.shape[1]
            tp = ps_tmp.tile([128, 512], F32, tag="pt")
            nc.tensor.transpose(tp[0:kdim, 0:B], inp, ident[0:B, 0:B])
            tsb = chk.tile([128, B], F32, tag=f"{tag}t")
            nc.scalar.activation(tsb[0:kdim, :], tp[0:kdim, 0:B], AF.Copy)
            ops = ps_tmp.tile([128, 512], F32, tag="pt")
            nc.tensor.matmul(
                ops[0:B, 0:ncols], tsb[0:kdim, 0:B], w_t[:], start=True, stop=False
            )
            nc.tensor.matmul(
                ops[0:B, 0:ncols], ones_lh[:, 0:B], b_t[:], start=False, stop=True
            )
            osb = chk.tile([B, max(ncols, 1)], F32, tag=f"{tag}o")
            nc.scalar.activation(osb[:, 0:ncols], ops[0:B, 0:ncols], act)
            return osb

        o1 = head_mm(pooled[:], small["fw1"], small["fb1"], AF.Relu, "o1")
        o2 = head_mm(o1[:], small["fw2"], small["fb2"], AF.Relu, "o2")
        o3 = head_mm(o2[:], small["fw3"], small["fb3"], AF.Copy, "o3")
        nc.sync.dma_start(out_y[:], o3[:, 0:1])


# ---------------------------------------------------------------- public API
def _in_maps(x, edge_index, batch, params):
    x = np.asarray(x, np.float32)
    graph = _prep_graph(np.asarray(edge_index))
    pools = _prep_pool(np.asarray(batch))
    p = _prep_params(params)
    xt = np.ascontiguousarray(x.T).astype(ml_dtypes.bfloat16)
    maps = []
    for c in range(NCORES):
        m = dict(
            xT=xt,
            xTl=np.ascontiguousarray(xt[:, c * NLOC : (c + 1) * NLOC]),
            src_idx=graph[c]["src_idx"],
            dst_idx=graph[c]["dst_idx"],
            s_mat=graph[c]["s_mat"].astype(ml_dtypes.bfloat16),
            pool_oh=pools[c],
        )
        for name in _WSHAPES:
            if name in m:
                continue
            arr = np.ascontiguousarray(p[name], np.float32)
            if name in BF_PARAMS:
                arr = arr.astype(ml_dtypes.bfloat16)
            m[name] = arr
        maps.append(m)
    return maps


def get_nc():
    if "nc" not in _CACHE:
        _CACHE["nc"] = _build()
    return _CACHE["nc"]


def kernel(x, edge_index, batch, params):
    nc = get_nc()
    res = run_bass_kernel_spmd(
        nc, _in_maps(x, edge_index, batch, params), list(range(NCORES))
    )
    return np.asarray(res.results[0]["y"], np.float32)
